# revision 2
# baseline (speedup 1.0000x reference)
"""MLA/GQA attention kernel v2 for Trainium2, 8-core SPMD.

Key wins over v1:
 - Latent-space attention: Q is projected straight into the 64-dim KV
   latent space (W_kf folded into W_q), K/V stay in latent space, and
   W_vf is folded into W_o.  Halves Q/K/V projection width; O-proj
   contracts head PAIRS (2x64=128) so its matmul count halves too.
 - bf16 operands everywhere (PSUM accumulation stays f32): halves DMA
   and SBUF, full PE rate at any free size.
 - Softmax denominator for free: v tiles are [g0 64 | ones 64 | g1 64];
   slot s takes columns s*64..s*64+128, so PV lands ctx on partitions
   s*64:(s+1)*64 (exactly where the O-proj pair tile wants it) and a
   64-way replicated denominator on the other half.  No den matmul.
 - qT kept resident in SBUF (no DRAM round trip).
 - O-projection interleaved into the attention instruction stream to
   fill PE bubbles left by the ACT-bound exp.

Sharding: core c = (batch b=c//2, head-half p=c%2): 8 q heads + 2 kv
groups per core; W_o row-sharded, partial outputs summed on host.
"""
import sys

sys.path.insert(0, "/opt/trn_rl_repo")

import numpy as np
import ml_dtypes

import concourse.bass as bass  # noqa: F401
import concourse.mybir as mybir
import concourse.tile as tile
from concourse import bacc, bass_utils

D = 2048
T = 2048
NH = 16
NKV = 4
DH = 128
LAT = 64
B = 4
NCORE = 8
HQ = 8            # q heads per core
NJ = 4            # q-pair tiles per core (tile j = heads j and j+4)
SCALE = 1.0 / np.sqrt(np.float32(DH))

NCC = D // 128    # 16 contraction chunks
NT = T // 128     # 16 key chunks
NQ = T // 512     # 4 query blocks

F32 = mybir.dt.float32
BF16 = mybir.dt.bfloat16
EXP = mybir.ActivationFunctionType.Exp

_CACHE = {}


def _build(reps=1):
    nc = bacc.Bacc("TRN2", target_bir_lowering=False, debug=False)
    xt_d = nc.dram_tensor("xt", [D, T], BF16, kind="ExternalInput").ap()
    wq_d = nc.dram_tensor("wq", [D, NJ * 128], BF16, kind="ExternalInput").ap()
    wk_d = nc.dram_tensor("wk", [D, 128], BF16, kind="ExternalInput").ap()
    wv_d = nc.dram_tensor("wv", [D, 128], BF16, kind="ExternalInput").ap()
    wo_d = nc.dram_tensor("wo", [NJ * 128, D], BF16, kind="ExternalInput").ap()
    out_d = nc.dram_tensor("out", [T, D], BF16, kind="ExternalOutput").ap()

    with tile.TileContext(nc) as tc:
      for rep in range(reps):
        R = f"r{rep}"
        with tc.tile_pool(name=f"persist{R}", bufs=1) as persist:
            # ---- persistent SBUF tensors ----
            xts = [persist.tile([128, T], BF16, tag=f"x{c}{R}", name=f"x{c}{R}")
                   for c in range(NCC)]
            wqs = [persist.tile([128, NJ * 128], BF16, tag=f"wq{c}{R}", name=f"wq{c}{R}")
                   for c in range(NCC)]
            wks = [persist.tile([128, 128], BF16, tag=f"wk{c}{R}", name=f"wk{c}{R}")
                   for c in range(NCC)]
            wvs = [persist.tile([128, 128], BF16, tag=f"wv{c}{R}", name=f"wv{c}{R}")
                   for c in range(NCC)]
            wos = [persist.tile([128, D], BF16, tag=f"wo{j}{R}", name=f"wo{j}{R}")
                   for j in range(NJ)]
            qts = [persist.tile([128, T], BF16, tag=f"q{j}{R}", name=f"q{j}{R}")
                   for j in range(NJ)]
            kt = persist.tile([128, T], BF16, tag=f"kt{R}", name=f"kt{R}")
            # v tiles: [g0 feats 64 | ones 64 | g1 feats 64]
            vts = [persist.tile([128, 192], BF16, tag=f"v{t}{R}", name=f"v{t}{R}")
                   for t in range(NT)]
            ctxp = [persist.tile([128, T], BF16, tag=f"c{j}{R}", name=f"c{j}{R}")
                    for j in range(NJ)]

            for c in range(NCC):
                nc.sync.dma_start(xts[c][:], xt_d[c * 128:(c + 1) * 128, :])
            for c in range(NCC):
                nc.sync.dma_start(wks[c][:], wk_d[c * 128:(c + 1) * 128, :])
                nc.sync.dma_start(wvs[c][:], wv_d[c * 128:(c + 1) * 128, :])
                nc.sync.dma_start(wqs[c][:], wq_d[c * 128:(c + 1) * 128, :])
            for j in range(NJ):
                nc.sync.dma_start(wos[j][:], wo_d[j * 128:(j + 1) * 128, :])
            for t in range(NT):
                nc.vector.memset(vts[t][:], 1.0)

            # ---------------- Phase P: projections ----------------
            with tc.tile_pool(name=f"pbig{R}", bufs=2, space="PSUM") as pbig:
                # K: k_lat^T [128, T]
                pk = pbig.tile([128, T], F32, tag="pb")
                for c in range(NCC):
                    for f in range(NQ):
                        nc.tensor.matmul(
                            pk[:, f * 512:(f + 1) * 512], wks[c][:],
                            xts[c][:, f * 512:(f + 1) * 512],
                            start=(c == 0), stop=(c == NCC - 1))
                for f in range(NQ):
                    nc.vector.tensor_copy(kt[:, f * 512:(f + 1) * 512],
                                          pk[:, f * 512:(f + 1) * 512])
                # V: natural orientation, 4 token chunks per big tile
                for r in range(NT // 4):
                    pv = pbig.tile([128, T], F32, tag="pb")
                    for c in range(NCC):
                        for tl in range(4):
                            tg = 4 * r + tl
                            nc.tensor.matmul(
                                pv[:, tl * 512:tl * 512 + 128],
                                xts[c][:, tg * 128:(tg + 1) * 128], wvs[c][:],
                                start=(c == 0), stop=(c == NCC - 1))
                    for tl in range(4):
                        tg = 4 * r + tl
                        nc.vector.tensor_copy(
                            vts[tg][:, 0:64], pv[:, tl * 512:tl * 512 + 64])
                        nc.vector.tensor_copy(
                            vts[tg][:, 128:192], pv[:, tl * 512 + 64:tl * 512 + 128])
                # Q: q_eff^T pair tiles [128, T]
                for j in range(NJ):
                    pq = pbig.tile([128, T], F32, tag="pb")
                    for c in range(NCC):
                        for f in range(NQ):
                            nc.tensor.matmul(
                                pq[:, f * 512:(f + 1) * 512],
                                wqs[c][:, j * 128:(j + 1) * 128],
                                xts[c][:, f * 512:(f + 1) * 512],
                                start=(c == 0), stop=(c == NCC - 1))
                    for f in range(NQ):
                        nc.vector.tensor_copy(qts[j][:, f * 512:(f + 1) * 512],
                                              pq[:, f * 512:(f + 1) * 512])

            # ---------------- Phase A+O ----------------
            with tc.tile_pool(name=f"as{R}", bufs=2, space="PSUM") as asp, \
                 tc.tile_pool(name=f"actx{R}", bufs=2, space="PSUM") as acp, \
                 tc.tile_pool(name=f"aoo{R}", bufs=2, space="PSUM") as aop, \
                 tc.tile_pool(name=f"aexp{R}", bufs=4) as aexp, \
                 tc.tile_pool(name=f"arec{R}", bufs=2) as arec, \
                 tc.tile_pool(name=f"aost{R}", bufs=2) as aost:

                pend = []          # deferred O-proj groups (tg, od)
                ostage = {}        # tg -> staging tile

                def emit_o(tg, od):
                    if od == 0:
                        ostage[tg] = aost.tile([128, D], BF16, tag="ost",
                                               name=f"ost{tg}{R}")
                    oo = aop.tile([128, 512], F32, tag="oo")
                    for j in range(NJ):
                        nc.tensor.matmul(
                            oo[:], ctxp[j][:, tg * 128:(tg + 1) * 128],
                            wos[j][:, od * 512:(od + 1) * 512],
                            start=(j == 0), stop=(j == NJ - 1))
                    st = ostage[tg]
                    nc.vector.tensor_copy(st[:, od * 512:(od + 1) * 512], oo[:])
                    if od == 3:
                        nc.sync.dma_start(
                            out_d[tg * 128:(tg + 1) * 128, :], st[:])
                        del ostage[tg]

                for qc in range(NQ):
                    for j in range(NJ):
                        for s in range(2):
                            ps_ctx = acp.tile([128, 512], F32, tag="ps_ctx")
                            for i in range(NT // 2):
                                ps_s = asp.tile([128, 1024], F32, tag="ps_s")
                                ex = aexp.tile([128, 1024], BF16, tag="exp")
                                for u in range(2):
                                    kc = 2 * i + u
                                    nc.tensor.matmul(
                                        ps_s[:, u * 512:(u + 1) * 512],
                                        kt[s * 64:(s + 1) * 64,
                                           kc * 128:(kc + 1) * 128],
                                        qts[j][s * 64:(s + 1) * 64,
                                               qc * 512:(qc + 1) * 512],
                                        start=True, stop=True)
                                nc.scalar.activation(ex[:], ps_s[:], EXP,
                                                     scale=float(SCALE))
                                for u in range(2):
                                    kc = 2 * i + u
                                    nc.tensor.matmul(
                                        ps_ctx[:],
                                        vts[kc][:, s * 64:s * 64 + 128],
                                        ex[:, u * 512:(u + 1) * 512],
                                        start=(kc == 0), stop=(kc == NT - 1))
                            # ctx on partitions s*64:(s+1)*64, den (64x
                            # replicated) on the other half
                            rec = arec.tile([128, 512], F32, tag="rec")
                            nc.vector.reciprocal(
                                rec[s * 64:(s + 1) * 64, :],
                                ps_ctx[(1 - s) * 64:(2 - s) * 64, :])
                            nc.vector.tensor_mul(
                                ctxp[j][s * 64:(s + 1) * 64,
                                        qc * 512:(qc + 1) * 512],
                                ps_ctx[s * 64:(s + 1) * 64, :],
                                rec[s * 64:(s + 1) * 64, :])
                            for _ in range(2):
                                if pend:
                                    emit_o(*pend.pop(0))
                    pend.extend((tg, od)
                                for tg in range(4 * qc, 4 * qc + 4)
                                for od in range(4))
                for g in pend:
                    emit_o(*g)

    nc.compile()
    return nc


LAST_RESULTS = None


def _prep_inputs(x, W_q, W_k, W_v, W_k_to_latent, W_v_to_latent,
                 W_k_from_latent, W_v_from_latent, W_o):
    x = np.asarray(x, np.float32)
    W_q = np.asarray(W_q, np.float32)
    W_k = np.asarray(W_k, np.float32)
    W_v = np.asarray(W_v, np.float32)
    W_ktl = np.asarray(W_k_to_latent, np.float32)
    W_vtl = np.asarray(W_v_to_latent, np.float32)
    W_kf = np.asarray(W_k_from_latent, np.float32)
    W_vf = np.asarray(W_v_from_latent, np.float32)
    W_o = np.asarray(W_o, np.float32)

    # fold W_kf into Q; keep K/V in latent space; fold W_vf into W_o
    wq_eff = np.stack([W_q[:, h * DH:(h + 1) * DH] @ W_kf.T
                       for h in range(NH)], 1)          # [D, NH, LAT]
    wk_lat = np.stack([W_k[:, g * DH:(g + 1) * DH] @ W_ktl
                       for g in range(NKV)], 1)         # [D, NKV, LAT]
    wv_lat = np.stack([W_v[:, g * DH:(g + 1) * DH] @ W_vtl
                       for g in range(NKV)], 1)
    wo_eff = np.stack([W_vf @ W_o[h * DH:(h + 1) * DH, :]
                       for h in range(NH)], 0)          # [NH, LAT, D]

    bf = ml_dtypes.bfloat16
    in_maps = []
    for c in range(NCORE):
        b, p = c // 2, c % 2
        heads = [8 * p + j for j in range(HQ)]
        # q columns: pair tile j = [head j, head j+4]
        wq_core = np.concatenate(
            [np.concatenate([wq_eff[:, heads[j]], wq_eff[:, heads[j + 4]]], 1)
             for j in range(NJ)], 1)                    # [D, 512]
        wk_core = np.concatenate([wk_lat[:, 2 * p], wk_lat[:, 2 * p + 1]], 1)
        wv_core = np.concatenate([wv_lat[:, 2 * p], wv_lat[:, 2 * p + 1]], 1)
        wo_core = np.concatenate(
            [np.concatenate([wo_eff[heads[j]], wo_eff[heads[j + 4]]], 0)
             for j in range(NJ)], 0)                    # [512, D]
        in_maps.append({
            "xt": np.ascontiguousarray(x[b].T).astype(bf),
            "wq": np.ascontiguousarray(wq_core).astype(bf),
            "wk": np.ascontiguousarray(wk_core).astype(bf),
            "wv": np.ascontiguousarray(wv_core).astype(bf),
            "wo": np.ascontiguousarray(wo_core).astype(bf),
        })
    return in_maps


def kernel(x, W_q, W_k, W_v, W_k_to_latent, W_v_to_latent,
           W_k_from_latent, W_v_from_latent, W_o):
    global LAST_RESULTS
    in_maps = _prep_inputs(x, W_q, W_k, W_v, W_k_to_latent, W_v_to_latent,
                           W_k_from_latent, W_v_from_latent, W_o)
    if "nc" not in _CACHE:
        _CACHE["nc"] = _build()
    nc = _CACHE["nc"]
    res = bass_utils.run_bass_kernel_spmd(nc, in_maps, core_ids=list(range(NCORE)))
    LAST_RESULTS = res
    out = np.empty((B, T, D), np.float32)
    for b in range(B):
        out[b] = (res.results[2 * b]["out"].astype(np.float32)
                  + res.results[2 * b + 1]["out"].astype(np.float32))
    return out
